# revision 1
# baseline (speedup 1.0000x reference)
"""Trainium2 Bass kernel for LMPNN-style GNN message passing + entity double-matmul.

Reference computation:
    msg      = (x[src] + rel_emb[rel]) * (1 - 2*neg)        # [E, D]
    aggr_out = segment_sum(msg, dst, N)                     # [N, D]
    aggr     = 0.1*x + aggr_out
    score    = relu((aggr @ E^T) * scale + bias)            # [N, V]
    out      = score @ E                                    # [N, D]

Strategy (8 NeuronCores, node-sharded, no collectives):
  * Core c owns nodes [c*512, (c+1)*512).
  * Message passing is re-expressed densely:  aggr = A @ x + R @ rel_emb,
    where A[n, m] = sum of (1-2*neg) over edges m->n  (+0.1 on the diagonal
    for the residual term) and R[n, r] = sum of (1-2*neg) over edges with
    relation r landing on n. The host builds the integer-valued A/R count
    matrices from the index tensors (pure index preprocessing); the device
    does all floating-point work as dense TensorEngine matmuls accumulated
    in fp32 PSUM, producing aggrT [D, 512] directly.
  * The double matmul streams the (host-transposed / host-swizzled) entity
    table from HBM in bf16, interleaving per-128-entity chunks:
    scoreT = ET_chunk(lhsT) x aggrT -> relu(+scale/bias) on ACT/DVE ->
    outT += E_chunk(lhsT) x scoreT accumulated in a single PSUM bank.
  * Output is outT [128, 512] fp32 per core; host transposes/concats.
"""

import sys

import numpy as np

try:
    import concourse.bass as bass
except ImportError:  # pragma: no cover
    sys.path.insert(0, "/opt/trn_rl_repo")
    import concourse.bass as bass

import ml_dtypes

import concourse.bacc as bacc
import concourse.mybir as mybir
import concourse.tile as tile
from concourse.bass_utils import run_bass_kernel_spmd

BF16 = ml_dtypes.bfloat16
F32 = np.float32


class Cfg:
    def __init__(self, N=4096, E=262144, D=128, R=1000, V=50000, C=8):
        self.N, self.E, self.D, self.R, self.V, self.C = N, E, D, R, V, C
        self.NPC = N // C                       # nodes per core
        assert self.NPC % 128 == 0 and N % 128 == 0
        self.RPAD = ((R + 127) // 128) * 128    # padded relation count
        self.VPAD = ((V + 511) // 512) * 512    # padded entity count
        self.NV = self.VPAD // 128              # 128-entity chunks
        self.NKX = N // 128                     # k-chunks for A @ x
        self.NKR = self.RPAD // 128             # k-chunks for R @ rel


def host_prep(cfg, x, edge_index, relation_id, neg_flag, rel_emb, entity_emb,
              scale, bias):
    """Build per-core in_maps. The host only converts the edge/index tensors
    into dense count matrices + does layout/dtype conversion; all FP math on
    the embeddings happens on device."""
    src = np.asarray(edge_index[0]).astype(np.int64)
    dst = np.asarray(edge_index[1]).astype(np.int64)
    rel = np.asarray(relation_id).astype(np.int64)
    neg = np.asarray(neg_flag).astype(np.int64)
    x = np.asarray(x, F32)
    rel_emb = np.asarray(rel_emb, F32)
    entity_emb = np.asarray(entity_emb, F32)
    scale = np.asarray(scale, F32)
    bias = np.asarray(bias, F32)

    C, NPC, D = cfg.C, cfg.NPC, cfg.D
    negc = (1.0 - 2.0 * neg).astype(F32)

    # dense message-passing operators (index preprocessing)
    A = np.zeros((cfg.N, cfg.N), F32)
    np.add.at(A, (dst, src), negc)
    A[np.arange(cfg.N), np.arange(cfg.N)] += 0.1          # residual 0.1*x
    Rm = np.zeros((cfg.N, cfg.RPAD), F32)
    np.add.at(Rm, (dst, rel), negc)

    # shared (replicated) tensors
    vpad = cfg.VPAD
    E_pad = np.zeros((vpad, D), F32)
    E_pad[: cfg.V] = entity_emb
    et_tab = np.ascontiguousarray(E_pad.T).astype(BF16)            # [128, VPAD]
    e_sw = np.ascontiguousarray(
        E_pad.reshape(vpad // 512, 4, 128, D).transpose(0, 2, 1, 3)
    ).astype(BF16)                                                 # [VPAD/512,128,4,D]
    scale_pad = np.ones(vpad, F32)
    scale_pad[: cfg.V] = scale
    bias_pad = np.zeros(vpad, F32)
    bias_pad[: cfg.V] = bias
    scaleT = np.ascontiguousarray(scale_pad.reshape(cfg.NV, 128).T)
    biasT = np.ascontiguousarray(bias_pad.reshape(cfg.NV, 128).T)
    fast_relu = bool(np.all(scale == 1.0) and np.all(bias == 0.0))

    xb = x.astype(BF16)                                            # [N, D]
    rb = np.zeros((cfg.RPAD, D), F32)
    rb[: cfg.R] = rel_emb
    rb = rb.astype(BF16)

    shared = {
        "x_b": xb, "rel_b": rb, "et_tab": et_tab, "e_sw": e_sw,
        "scaleT": scaleT, "biasT": biasT,
    }
    in_maps = []
    for c in range(C):
        rows = slice(c * NPC, (c + 1) * NPC)
        at_c = np.ascontiguousarray(A[rows].T).astype(BF16)        # [N, NPC]
        rt_c = np.ascontiguousarray(Rm[rows].T).astype(BF16)       # [RPAD, NPC]
        m = dict(shared)
        m.update({"a_t": at_c, "r_t": rt_c})
        in_maps.append(m)
    return in_maps, fast_relu


def build(cfg, fast_relu, enable_asserts=False, dve_mod=2, dve_thresh=1):
    f32, bf16 = mybir.dt.float32, mybir.dt.bfloat16
    nc = bacc.Bacc(
        "TRN2", target_bir_lowering=False, debug=False,
        enable_asserts=enable_asserts,
    )
    D, NPC, NV = cfg.D, cfg.NPC, cfg.NV

    xb_t = nc.dram_tensor("x_b", [cfg.N, D], bf16, kind="ExternalInput").ap()
    rb_t = nc.dram_tensor("rel_b", [cfg.RPAD, D], bf16, kind="ExternalInput").ap()
    at_t = nc.dram_tensor("a_t", [cfg.N, NPC], bf16, kind="ExternalInput").ap()
    rt_t = nc.dram_tensor("r_t", [cfg.RPAD, NPC], bf16, kind="ExternalInput").ap()
    ett_t = nc.dram_tensor("et_tab", [128, cfg.VPAD], bf16, kind="ExternalInput").ap()
    esw_t = nc.dram_tensor("e_sw", [cfg.VPAD // 512, 128, 4, D], bf16, kind="ExternalInput").ap()
    scl_t = nc.dram_tensor("scaleT", [128, NV], f32, kind="ExternalInput").ap()
    bia_t = nc.dram_tensor("biasT", [128, NV], f32, kind="ExternalInput").ap()
    out_t = nc.dram_tensor("out", [128, NPC], f32, kind="ExternalOutput").ap()

    Relu = mybir.ActivationFunctionType.Relu

    with tile.TileContext(nc) as tc:
        with (
            tc.tile_pool(name="const", bufs=1) as constp,
            tc.tile_pool(name="aggk", bufs=4) as akp,
            tc.tile_pool(name="etab", bufs=6) as ep,
            tc.tile_pool(name="scoresb", bufs=6) as scp,
            tc.tile_pool(name="psA", bufs=1, space="PSUM") as psA,
            tc.tile_pool(name="psS", bufs=6, space="PSUM") as psS,
            tc.tile_pool(name="psO", bufs=1, space="PSUM") as psO,
        ):
            sclt = constp.tile([128, NV], f32, tag="sc")
            nc.sync.dma_start(sclt, scl_t)
            biat = constp.tile([128, NV], f32, tag="bi")
            nc.sync.dma_start(biat, bia_t)
            aggrT_sb = constp.tile([128, NPC], bf16, tag="aggrT")
            out_sb = constp.tile([128, NPC], f32, tag="outsb")

            # ---- phase 1: aggrT = x^T A^T + rel^T R^T  (k-chunked) --------
            aggr_ps = psA.tile([128, NPC], f32, tag="aggrps")
            for k in range(cfg.NKX):
                ks = slice(k * 128, (k + 1) * 128)
                xk = akp.tile([128, D], bf16, tag="lhs")
                nc.sync.dma_start(xk, xb_t[ks, :])
                ak = akp.tile([128, NPC], bf16, tag="rhs")
                nc.sync.dma_start(ak, at_t[ks, :])
                nc.tensor.matmul(
                    aggr_ps, lhsT=xk, rhs=ak,
                    start=(k == 0), stop=False, skip_group_check=True,
                )
            for k in range(cfg.NKR):
                ks = slice(k * 128, (k + 1) * 128)
                rk = akp.tile([128, D], bf16, tag="lhs")
                nc.sync.dma_start(rk, rb_t[ks, :])
                rrk = akp.tile([128, NPC], bf16, tag="rhs")
                nc.sync.dma_start(rrk, rt_t[ks, :])
                nc.tensor.matmul(
                    aggr_ps, lhsT=rk, rhs=rrk,
                    start=False, stop=(k == cfg.NKR - 1), skip_group_check=True,
                )
            nc.vector.tensor_copy(aggrT_sb, aggr_ps)

            # ---- phase 2: fused double matmul over entity chunks ----------
            outT_ps = psO.tile([128, NPC], f32, tag="outps")
            for vb in range(cfg.VPAD // 512):
                ett = ep.tile([128, 512], bf16, tag="et")
                nc.sync.dma_start(ett, ett_t[:, vb * 512 : (vb + 1) * 512])
                esw = ep.tile([128, 4, D], bf16, tag="ee")
                nc.sync.dma_start(esw, esw_t[vb])
                for j in range(4):
                    v = vb * 4 + j
                    sps = psS.tile([128, NPC], f32, tag="sps")
                    nc.tensor.matmul(
                        sps, lhsT=ett[:, j * 128 : (j + 1) * 128], rhs=aggrT_sb,
                        start=True, stop=True, skip_group_check=True,
                    )
                    st_sb = scp.tile([128, NPC], bf16, tag="st")
                    if fast_relu:
                        if v % dve_mod < dve_thresh:
                            nc.vector.tensor_relu(st_sb, sps)
                        else:
                            nc.scalar.activation(st_sb, sps, Relu)
                    else:
                        nc.scalar.activation(
                            st_sb, sps, Relu,
                            bias=biat[:, v : v + 1], scale=sclt[:, v : v + 1],
                        )
                    nc.tensor.matmul(
                        outT_ps, lhsT=esw[:, j, :], rhs=st_sb,
                        start=(v == 0), stop=(v == NV - 1), skip_group_check=True,
                    )

            nc.vector.tensor_copy(out_sb, outT_ps)
            nc.sync.dma_start(out_t, out_sb)

    nc.compile()
    return nc


def run(inputs, trace=False, cfg=None, dve_mod=2, dve_thresh=1):
    if cfg is None:
        cfg = Cfg()
    in_maps, fast_relu = host_prep(cfg, **inputs)
    nc = build(cfg, fast_relu, dve_mod=dve_mod, dve_thresh=dve_thresh)
    try:
        res = run_bass_kernel_spmd(
            nc, in_maps, core_ids=list(range(cfg.C)), trace=trace,
        )
    except ModuleNotFoundError:
        # NTFF profiling hook unavailable in this container; run untraced.
        res = run_bass_kernel_spmd(
            nc, in_maps, core_ids=list(range(cfg.C)), trace=False,
        )
    outs = []
    for c in range(cfg.C):
        outs.append(np.ascontiguousarray(np.asarray(res.results[c]["out"]).T))
    full = np.concatenate(outs, axis=0).astype(np.float32)
    return full, res


def kernel(**inputs):
    full, _ = run(inputs, trace=False)
    return full



# revision 5
# speedup vs baseline: 1.4889x; 1.4889x over previous
"""Trainium2 Bass kernel for LMPNN-style GNN message passing + entity double-matmul.

Reference computation:
    msg      = (x[src] + rel_emb[rel]) * (1 - 2*neg)        # [E, D]
    aggr_out = segment_sum(msg, dst, N)                     # [N, D]
    aggr     = 0.1*x + aggr_out
    score    = relu((aggr @ E^T) * scale + bias)            # [N, V]
    out      = score @ E                                    # [N, D]

Strategy (8 NeuronCores, node-sharded, no collectives):
  * Core c owns nodes [c*512, (c+1)*512).
  * Message passing is re-expressed densely:  aggr = A @ x + R @ rel_emb,
    where A[n, m] = sum of (1-2*neg) over edges m->n  (+0.1 on the diagonal
    for the residual term) and R[n, r] the same per relation. The count
    matrices are small integers (|count| <= ~6) so they are streamed in
    fp8e4 (exact), halving phase-1 HBM traffic; x/rel_emb stay bf16 as the
    stationary matmul operands.
  * Double matmul: mm1 (score^T = ssc*E @ aggr^T) runs in bf16; relu converts
    PSUM fp32 -> fp8e4 SBUF tiles shaped [128, 2, 512]; mm2 accumulates
    out^T += (esc*E)^T @ score8 using fp8 DoubleRow matmuls (256-entity
    contraction per matmul). Scales ssc/esc are folded into the entity-table
    copies on the host (calibrated from row norms so fp8 can't overflow);
    the device divides them back out in the final PSUM->SBUF copy.
  * Relu work is spread across ACT/DVE(/GPSIMD) with a weighted schedule and
    the dependent mm2 is software-pipelined LAG groups behind mm1 so the
    TensorEngine never waits on relu latency.
  * Output is outT [128, 512] fp32 per core; host transposes/concats.
"""

import sys

import numpy as np

try:
    import concourse.bass as bass
except ImportError:  # pragma: no cover
    sys.path.insert(0, "/opt/trn_rl_repo")
    import concourse.bass as bass

import ml_dtypes

import concourse.bacc as bacc
import concourse.mybir as mybir
import concourse.tile as tile
from concourse.bass_utils import run_bass_kernel_spmd

BF16 = ml_dtypes.bfloat16
F8 = ml_dtypes.float8_e4m3  # matches mybir.dt.float8e4 (max finite 240)
F32 = np.float32

F8_MAX = 240.0
F8_MARGIN = 0.98


class Cfg:
    def __init__(self, N=4096, E=262144, D=128, R=1000, V=50000, C=8):
        self.N, self.E, self.D, self.R, self.V, self.C = N, E, D, R, V, C
        self.NPC = N // C                       # nodes per core
        assert self.NPC % 128 == 0 and N % 128 == 0
        self.RPAD = ((R + 127) // 128) * 128    # padded relation count
        self.VPAD = ((V + 1023) // 1024) * 1024  # padded entity count
        self.NV = self.VPAD // 128              # 128-entity chunks
        self.NG = self.VPAD // 256              # DoubleRow groups
        self.NT = self.VPAD // 1024             # 1024-entity stream tiles
        self.NKX = N // 128                     # k-chunks for A @ x
        self.NKR = self.RPAD // 128             # k-chunks for R @ rel
        self.XB = self.NKX // 2                 # x DMA blocks (2 chunks each)
        self.RB = self.NKR // 2


def _pm_pack(mat, nblk, width):
    """[nblk*2*128, width] -> partition-major [128, nblk, 2*width]."""
    r = mat.reshape(nblk, 2, 128, width).transpose(2, 0, 1, 3)
    return np.ascontiguousarray(r.reshape(128, nblk, 2 * width))


def host_prep(cfg, x, edge_index, relation_id, neg_flag, rel_emb, entity_emb,
              scale, bias):
    """Build per-core in_maps. The host converts the edge/index tensors into
    dense count matrices, lays tensors out partition-major for wide DMA
    descriptors, and calibrates the fp8 scales; the FP pipeline on the
    embeddings runs on device."""
    src = np.asarray(edge_index[0]).astype(np.int64)
    dst = np.asarray(edge_index[1]).astype(np.int64)
    rel = np.asarray(relation_id).astype(np.int64)
    neg = np.asarray(neg_flag).astype(np.int64)
    x = np.asarray(x, F32)
    rel_emb = np.asarray(rel_emb, F32)
    entity_emb = np.asarray(entity_emb, F32)
    scale = np.asarray(scale, F32)
    bias = np.asarray(bias, F32)

    C, NPC, D = cfg.C, cfg.NPC, cfg.D
    negc = (1.0 - 2.0 * neg).astype(F32)

    # dense message-passing operators (index preprocessing)
    A = np.zeros((cfg.N, cfg.N), F32)
    np.add.at(A, (dst, src), negc)
    A[np.arange(cfg.N), np.arange(cfg.N)] += 0.1          # residual 0.1*x
    Rm = np.zeros((cfg.N, cfg.RPAD), F32)
    np.add.at(Rm, (dst, rel), negc)

    relpad = np.zeros((cfg.RPAD, D), F32)
    relpad[: cfg.R] = rel_emb

    vpad = cfg.VPAD
    E_pad = np.zeros((vpad, D), F32)
    E_pad[: cfg.V] = entity_emb

    fast_relu = bool(np.all(scale == 1.0) and np.all(bias == 0.0))
    if not fast_relu:
        return _host_prep_general(cfg, x, A, Rm, relpad, E_pad, scale, bias)

    # fp8 scale calibration: score = aggr @ E^T bounded by row-norm products.
    aggr_h = A @ x + Rm @ relpad
    sbound = np.linalg.norm(aggr_h, axis=1).max() * \
        np.linalg.norm(E_pad, axis=1).max()
    ssc = F8_MAX * F8_MARGIN / max(sbound, 1e-30)
    esc = F8_MAX * F8_MARGIN / max(np.abs(E_pad).max(), 1e-30)
    asc = F8_MAX * 0.95 / max(np.abs(aggr_h).max(), 1e-30)
    col = lambda v: np.full((128, 1), v, F32)

    # shared (replicated) tensors
    et8 = ((E_pad.T * esc)).astype(F8)                             # [128, VPAD]
    ett8 = np.ascontiguousarray(
        np.broadcast_to(
            et8.reshape(128, cfg.NT, 1, 1024), (128, cfg.NT, 2, 1024)
        )
    )                                                              # [128, NT, 2, 1024]
    esw8 = np.ascontiguousarray(
        (E_pad * esc).reshape(cfg.NT, 8, 128, D).transpose(2, 0, 1, 3)
    ).astype(F8)                                                   # [128, NT, 8, 128]
    x_pm = _pm_pack(x, cfg.XB, D).astype(BF16)                     # [128, XB, 256]
    rel_pm = _pm_pack(relpad, cfg.RB, D).astype(BF16)              # [128, RB, 256]

    shared = {
        "x_pm": x_pm, "rel_pm": rel_pm, "ett8": ett8, "esw8": esw8,
        "asc": col(asc), "krelu": col(ssc / (esc * asc)),
        "descale": col(1.0 / (ssc * esc)),
    }
    in_maps = []
    for c in range(C):
        rows = slice(c * NPC, (c + 1) * NPC)
        a_pm = _pm_pack(np.ascontiguousarray(A[rows].T), cfg.XB, NPC).astype(F8)
        r_pm = _pm_pack(np.ascontiguousarray(Rm[rows].T), cfg.RB, NPC).astype(F8)
        m = dict(shared)
        m.update({"a_pm": a_pm, "r_pm": r_pm})
        in_maps.append(m)
    return in_maps, fast_relu


def build(cfg, fast_relu, lag=2, relu_cost=(1038.0, 1192.0)):
    if not fast_relu:
        return _build_general(cfg, fast_relu)

    f32, bf16, fp8 = mybir.dt.float32, mybir.dt.bfloat16, mybir.dt.float8e4
    nc = bacc.Bacc("TRN2", target_bir_lowering=False, debug=False)
    D, NPC, NG, NT = cfg.D, cfg.NPC, cfg.NG, cfg.NT

    xp_t = nc.dram_tensor("x_pm", [128, cfg.XB, 2 * D], bf16, kind="ExternalInput").ap()
    rp_t = nc.dram_tensor("rel_pm", [128, cfg.RB, 2 * D], bf16, kind="ExternalInput").ap()
    ap_t = nc.dram_tensor("a_pm", [128, cfg.XB, 2 * NPC], fp8, kind="ExternalInput").ap()
    rr_t = nc.dram_tensor("r_pm", [128, cfg.RB, 2 * NPC], fp8, kind="ExternalInput").ap()
    ett_t = nc.dram_tensor("ett8", [128, NT, 2, 1024], fp8, kind="ExternalInput").ap()
    esw_t = nc.dram_tensor("esw8", [128, NT, 8, D], fp8, kind="ExternalInput").ap()
    asc_t = nc.dram_tensor("asc", [128, 1], f32, kind="ExternalInput").ap()
    krl_t = nc.dram_tensor("krelu", [128, 1], f32, kind="ExternalInput").ap()
    dsc_t = nc.dram_tensor("descale", [128, 1], f32, kind="ExternalInput").ap()
    out_t = nc.dram_tensor("out", [128, NPC], f32, kind="ExternalOutput").ap()

    Relu = mybir.ActivationFunctionType.Relu
    Copy = mybir.ActivationFunctionType.Copy
    DR = mybir.MatmulPerfMode.DoubleRow

    # weighted relu schedule across ACT / DVE by per-group cost
    engines = [0, 1]
    load = {e: 0.0 for e in engines}
    sched = []
    for _ in range(NG):
        e = min(engines, key=lambda i: load[i] + relu_cost[i])
        load[e] += relu_cost[e]
        sched.append(e)

    with tile.TileContext(nc) as tc:
        with (
            tc.tile_pool(name="const", bufs=1) as constp,
            tc.tile_pool(name="p1x", bufs=2) as p1x,
            tc.tile_pool(name="p1a", bufs=2) as p1a,
            tc.tile_pool(name="etab", bufs=4) as ep,
            tc.tile_pool(name="eswp", bufs=4) as sp,
            tc.tile_pool(name="scoresb", bufs=6) as scp,
            tc.tile_pool(name="psAcc", bufs=1, space="PSUM") as psAcc,
            tc.tile_pool(name="psS", bufs=3, space="PSUM") as psS,
        ):
            asc_sb = constp.tile([128, 1], f32, tag="asc")
            nc.sync.dma_start(asc_sb, asc_t)
            krl_sb = constp.tile([128, 1], f32, tag="krl")
            nc.sync.dma_start(krl_sb, krl_t)
            dsc_sb = constp.tile([128, 1], f32, tag="dsc")
            nc.sync.dma_start(dsc_sb, dsc_t)
            aggr_s = constp.tile([128, NPC], f32, tag="aggrs")
            aggrT8 = constp.tile([128, 2, NPC], fp8, tag="aggrT8")
            out_sb = constp.tile([128, NPC], f32, tag="outsb")

            # ---- phase 1: aggrT = x^T A^T + rel^T R^T  (k-chunked) --------
            aggr_ps = psAcc.tile([128, NPC], f32, tag="accps")
            nblk = 4  # x/a DMA granularity in 2-chunk blocks
            first = True
            for lhs_t, rhs_t, NB in ((xp_t, ap_t, cfg.XB), (rp_t, rr_t, cfg.RB)):
                for b0 in range(0, NB, nblk):
                    nb = min(nblk, NB - b0)
                    lt = p1x.tile([128, nb, 2 * D], bf16, tag="lhs")
                    nc.sync.dma_start(lt, lhs_t[:, b0 : b0 + nb, :])
                    rt = p1a.tile([128, nb, 2 * NPC], fp8, tag="rhs")
                    nc.sync.dma_start(rt, rhs_t[:, b0 : b0 + nb, :])
                    for bb in range(nb):
                        for h in range(2):
                            last = (
                                rhs_t is rr_t
                                and b0 + bb == NB - 1
                                and h == 1
                            )
                            nc.tensor.matmul(
                                aggr_ps,
                                lhsT=lt[:, bb, h * D : (h + 1) * D],
                                rhs=rt[:, bb, h * NPC : (h + 1) * NPC],
                                start=first, stop=last, skip_group_check=True,
                            )
                            first = False

            # aggrT8 = hi/lo fp8 split of asc*aggrT (one shared scale, so the
            # DoubleRow planes can be summed raw by the PE)
            nc.vector.tensor_scalar_mul(aggr_s, aggr_ps, asc_sb)
            nc.scalar.activation(aggrT8[:, 0, :], aggr_s, Copy)
            nc.vector.tensor_sub(aggrT8[:, 1, :], aggr_s, aggrT8[:, 0, :])

            # ---- phase 2: fused double matmul over entity chunks ----------
            outT_ps = psAcc.tile([128, NPC], f32, tag="outps")
            pend = []  # software-pipelined DoubleRow mm2s: (g, esw_tile, st8)

            def flush_mm2():
                g, esw_g, st8_g = pend.pop(0)
                nc.tensor.matmul(
                    outT_ps, lhsT=esw_g, rhs=st8_g,
                    start=(g == 0), stop=(g == NG - 1),
                    perf_mode=DR, skip_group_check=True,
                )

            for tb in range(NT):
                ett_tile = ep.tile([128, 2, 1024], fp8, tag="et")
                nc.sync.dma_start(ett_tile, ett_t[:, tb])
                esw_tile = sp.tile([128, 8, D], fp8, tag="ee")
                nc.sync.dma_start(esw_tile, esw_t[:, tb])
                for u in range(4):
                    g = 4 * tb + u
                    sps = psS.tile([128, 1024], f32, tag="sps")
                    for h in range(2):
                        nc.tensor.matmul(
                            sps[:, h * NPC : (h + 1) * NPC],
                            lhsT=ett_tile[:, :, (2 * u + h) * 128 : (2 * u + h + 1) * 128],
                            rhs=aggrT8,
                            start=True, stop=True,
                            perf_mode=DR, skip_group_check=True,
                        )
                    st8 = scp.tile([128, 2, NPC], fp8, tag="st")
                    if sched[g] == 0:
                        nc.scalar.activation(st8, sps, Relu, scale=krl_sb)
                    else:
                        nc.vector.tensor_scalar(
                            st8, sps, krl_sb, 0.0,
                            op0=mybir.AluOpType.mult, op1=mybir.AluOpType.max,
                        )
                    pend.append((g, esw_tile[:, 2 * u : 2 * u + 2, :], st8))
                    if len(pend) > lag:
                        flush_mm2()
            while pend:
                flush_mm2()

            nc.vector.tensor_scalar_mul(out_sb, outT_ps, dsc_sb)
            nc.sync.dma_start(out_t, out_sb)

    nc.compile()
    return nc


# ---------------------------------------------------------------------------
# General-path fallback (non-trivial scale/bias): previous bf16 kernel.
# ---------------------------------------------------------------------------

def _host_prep_general(cfg, x, A, Rm, relpad, E_pad, scale, bias):
    vpad = cfg.VPAD
    et_tab = np.ascontiguousarray(E_pad.T).astype(BF16)            # [128, VPAD]
    e_sw = np.ascontiguousarray(
        E_pad.reshape(vpad // 512, 4, 128, cfg.D).transpose(0, 2, 1, 3)
    ).astype(BF16)
    scale_pad = np.ones(vpad, F32)
    scale_pad[: cfg.V] = scale
    bias_pad = np.zeros(vpad, F32)
    bias_pad[: cfg.V] = bias
    scaleT = np.ascontiguousarray(scale_pad.reshape(cfg.NV, 128).T)
    biasT = np.ascontiguousarray(bias_pad.reshape(cfg.NV, 128).T)

    xb = x.astype(BF16)
    rb = relpad.astype(BF16)

    shared = {
        "x_b": xb, "rel_b": rb, "et_tab": et_tab, "e_sw": e_sw,
        "scaleT": scaleT, "biasT": biasT,
    }
    in_maps = []
    for c in range(cfg.C):
        rows = slice(c * cfg.NPC, (c + 1) * cfg.NPC)
        at_c = np.ascontiguousarray(A[rows].T).astype(BF16)
        rt_c = np.ascontiguousarray(Rm[rows].T).astype(BF16)
        m = dict(shared)
        m.update({"a_t": at_c, "r_t": rt_c})
        in_maps.append(m)
    return in_maps, False


def _build_general(cfg, fast_relu):
    f32, bf16 = mybir.dt.float32, mybir.dt.bfloat16
    nc = bacc.Bacc("TRN2", target_bir_lowering=False, debug=False)
    D, NPC, NV = cfg.D, cfg.NPC, cfg.NV

    xb_t = nc.dram_tensor("x_b", [cfg.N, D], bf16, kind="ExternalInput").ap()
    rb_t = nc.dram_tensor("rel_b", [cfg.RPAD, D], bf16, kind="ExternalInput").ap()
    at_t = nc.dram_tensor("a_t", [cfg.N, NPC], bf16, kind="ExternalInput").ap()
    rt_t = nc.dram_tensor("r_t", [cfg.RPAD, NPC], bf16, kind="ExternalInput").ap()
    ett_t = nc.dram_tensor("et_tab", [128, cfg.VPAD], bf16, kind="ExternalInput").ap()
    esw_t = nc.dram_tensor("e_sw", [cfg.VPAD // 512, 128, 4, D], bf16, kind="ExternalInput").ap()
    scl_t = nc.dram_tensor("scaleT", [128, NV], f32, kind="ExternalInput").ap()
    bia_t = nc.dram_tensor("biasT", [128, NV], f32, kind="ExternalInput").ap()
    out_t = nc.dram_tensor("out", [128, NPC], f32, kind="ExternalOutput").ap()

    Relu = mybir.ActivationFunctionType.Relu

    with tile.TileContext(nc) as tc:
        with (
            tc.tile_pool(name="const", bufs=1) as constp,
            tc.tile_pool(name="aggk", bufs=4) as akp,
            tc.tile_pool(name="etab", bufs=6) as ep,
            tc.tile_pool(name="scoresb", bufs=6) as scp,
            tc.tile_pool(name="psA", bufs=1, space="PSUM") as psA,
            tc.tile_pool(name="psS", bufs=6, space="PSUM") as psS,
            tc.tile_pool(name="psO", bufs=1, space="PSUM") as psO,
        ):
            sclt = constp.tile([128, NV], f32, tag="sc")
            nc.sync.dma_start(sclt, scl_t)
            biat = constp.tile([128, NV], f32, tag="bi")
            nc.sync.dma_start(biat, bia_t)
            aggrT_sb = constp.tile([128, NPC], bf16, tag="aggrT")
            out_sb = constp.tile([128, NPC], f32, tag="outsb")

            aggr_ps = psA.tile([128, NPC], f32, tag="aggrps")
            for k in range(cfg.NKX):
                ks = slice(k * 128, (k + 1) * 128)
                xk = akp.tile([128, D], bf16, tag="lhs")
                nc.sync.dma_start(xk, xb_t[ks, :])
                ak = akp.tile([128, NPC], bf16, tag="rhs")
                nc.sync.dma_start(ak, at_t[ks, :])
                nc.tensor.matmul(
                    aggr_ps, lhsT=xk, rhs=ak,
                    start=(k == 0), stop=False, skip_group_check=True,
                )
            for k in range(cfg.NKR):
                ks = slice(k * 128, (k + 1) * 128)
                rk = akp.tile([128, D], bf16, tag="lhs")
                nc.sync.dma_start(rk, rb_t[ks, :])
                rrk = akp.tile([128, NPC], bf16, tag="rhs")
                nc.sync.dma_start(rrk, rt_t[ks, :])
                nc.tensor.matmul(
                    aggr_ps, lhsT=rk, rhs=rrk,
                    start=False, stop=(k == cfg.NKR - 1), skip_group_check=True,
                )
            nc.vector.tensor_copy(aggrT_sb, aggr_ps)

            outT_ps = psO.tile([128, NPC], f32, tag="outps")
            for vb in range(cfg.VPAD // 512):
                ett = ep.tile([128, 512], bf16, tag="et")
                nc.sync.dma_start(ett, ett_t[:, vb * 512 : (vb + 1) * 512])
                esw = ep.tile([128, 4, D], bf16, tag="ee")
                nc.sync.dma_start(esw, esw_t[vb])
                for j in range(4):
                    v = vb * 4 + j
                    sps = psS.tile([128, NPC], f32, tag="sps")
                    nc.tensor.matmul(
                        sps, lhsT=ett[:, j * 128 : (j + 1) * 128], rhs=aggrT_sb,
                        start=True, stop=True, skip_group_check=True,
                    )
                    st_sb = scp.tile([128, NPC], bf16, tag="st")
                    nc.scalar.activation(
                        st_sb, sps, Relu,
                        bias=biat[:, v : v + 1], scale=sclt[:, v : v + 1],
                    )
                    nc.tensor.matmul(
                        outT_ps, lhsT=esw[:, j, :], rhs=st_sb,
                        start=(v == 0), stop=(v == NV - 1), skip_group_check=True,
                    )

            nc.vector.tensor_copy(out_sb, outT_ps)
            nc.sync.dma_start(out_t, out_sb)

    nc.compile()
    return nc


def run(inputs, trace=False, cfg=None, **build_kwargs):
    if cfg is None:
        cfg = Cfg()
    in_maps, fast_relu = host_prep(cfg, **inputs)
    nc = build(cfg, fast_relu, **build_kwargs)
    try:
        res = run_bass_kernel_spmd(
            nc, in_maps, core_ids=list(range(cfg.C)), trace=trace,
        )
    except ModuleNotFoundError:
        # NTFF profiling hook unavailable in this container; run untraced.
        res = run_bass_kernel_spmd(
            nc, in_maps, core_ids=list(range(cfg.C)), trace=False,
        )
    outs = []
    for c in range(cfg.C):
        outs.append(np.ascontiguousarray(np.asarray(res.results[c]["out"]).T))
    full = np.concatenate(outs, axis=0).astype(np.float32)
    return full, res


def kernel(**inputs):
    full, _ = run(inputs, trace=False)
    return full


# revision 27
# speedup vs baseline: 1.6071x; 1.0794x over previous
"""Trainium2 Bass kernel for LMPNN-style GNN message passing + entity double-matmul.

Reference computation:
    msg      = (x[src] + rel_emb[rel]) * (1 - 2*neg)        # [E, D]
    aggr_out = segment_sum(msg, dst, N)                     # [N, D]
    aggr     = 0.1*x + aggr_out
    score    = relu((aggr @ E^T) * scale + bias)            # [N, V]
    out      = score @ E                                    # [N, D]

Strategy (8 NeuronCores, node-sharded, no collectives):
  * Core c owns nodes [c*512, (c+1)*512).
  * Message passing is re-expressed densely:  aggr = A @ x + R @ rel_emb,
    where A[n, m] = sum of (1-2*neg) over edges m->n  (+0.1 on the diagonal
    for the residual term) and R[n, r] the same per relation. The count
    matrices are small integers (|count| <= ~6) so they are streamed in
    fp8e4 (exact), halving phase-1 HBM traffic; x/rel_emb stay bf16 as the
    stationary matmul operands.
  * Double matmul: mm1 (score^T = ssc*E @ aggr^T) runs in bf16; relu converts
    PSUM fp32 -> fp8e4 SBUF tiles shaped [128, 2, 512]; mm2 accumulates
    out^T += (esc*E)^T @ score8 using fp8 DoubleRow matmuls (256-entity
    contraction per matmul). Scales ssc/esc are folded into the entity-table
    copies on the host (calibrated from row norms so fp8 can't overflow);
    the device divides them back out in the final PSUM->SBUF copy.
  * Relu work is spread across ACT/DVE(/GPSIMD) with a weighted schedule and
    the dependent mm2 is software-pipelined LAG groups behind mm1 so the
    TensorEngine never waits on relu latency.
  * Output is outT [128, 512] fp32 per core; host transposes/concats.
"""

import sys

import numpy as np

try:
    import concourse.bass as bass
except ImportError:  # pragma: no cover
    sys.path.insert(0, "/opt/trn_rl_repo")
    import concourse.bass as bass

import ml_dtypes

import concourse.bacc as bacc
import concourse.mybir as mybir
import concourse.tile as tile
from concourse.bass_utils import run_bass_kernel_spmd

BF16 = ml_dtypes.bfloat16
F8 = ml_dtypes.float8_e4m3  # matches mybir.dt.float8e4 (max finite 240)
F32 = np.float32

F8_MAX = 240.0
F8_MARGIN = 0.98


class Cfg:
    def __init__(self, N=4096, E=262144, D=128, R=1000, V=50000, C=8):
        self.N, self.E, self.D, self.R, self.V, self.C = N, E, D, R, V, C
        self.NPC = N // C                       # nodes per core
        assert self.NPC % 128 == 0 and N % 128 == 0
        self.RPAD = ((R + 127) // 128) * 128    # padded relation count
        self.VPAD = ((V + 1023) // 1024) * 1024  # padded entity count
        self.NV = self.VPAD // 128              # 128-entity chunks
        self.NG = self.VPAD // 256              # DoubleRow groups
        self.NT = self.VPAD // 1024             # 1024-entity stream tiles
        self.NKX = N // 128                     # k-chunks for A @ x
        self.NKR = self.RPAD // 128             # k-chunks for R @ rel
        self.XB = self.NKX // 2                 # x DMA blocks (2 chunks each)
        self.RB = self.NKR // 2


def _pm_pack(mat, nblk, width):
    """[nblk*128, width] -> partition-major [128, nblk, width]."""
    r = mat.reshape(nblk, 128, width).transpose(1, 0, 2)
    return np.ascontiguousarray(r)


def _hilo8(mat, sc):
    """fp8 hi/lo split of sc*mat -> [rows, 2, cols] (one shared scale)."""
    s = (mat * sc).astype(F32)
    hi = s.astype(F8)
    lo = (s - hi.astype(F32)).astype(F8)
    return np.stack([hi, lo], axis=1)


def host_prep(cfg, x, edge_index, relation_id, neg_flag, rel_emb, entity_emb,
              scale, bias):
    """Build per-core in_maps. The host converts the edge/index tensors into
    dense count matrices, lays tensors out partition-major for wide DMA
    descriptors, and calibrates the fp8 scales; the FP pipeline on the
    embeddings runs on device."""
    src = np.asarray(edge_index[0]).astype(np.int64)
    dst = np.asarray(edge_index[1]).astype(np.int64)
    rel = np.asarray(relation_id).astype(np.int64)
    neg = np.asarray(neg_flag).astype(np.int64)
    x = np.asarray(x, F32)
    rel_emb = np.asarray(rel_emb, F32)
    entity_emb = np.asarray(entity_emb, F32)
    scale = np.asarray(scale, F32)
    bias = np.asarray(bias, F32)

    C, NPC, D = cfg.C, cfg.NPC, cfg.D
    negc = (1.0 - 2.0 * neg).astype(F32)

    # dense message-passing operators (index preprocessing)
    A = np.zeros((cfg.N, cfg.N), F32)
    np.add.at(A, (dst, src), negc)
    A[np.arange(cfg.N), np.arange(cfg.N)] += 0.1          # residual 0.1*x
    Rm = np.zeros((cfg.N, cfg.RPAD), F32)
    np.add.at(Rm, (dst, rel), negc)

    relpad = np.zeros((cfg.RPAD, D), F32)
    relpad[: cfg.R] = rel_emb

    vpad = cfg.VPAD
    E_pad = np.zeros((vpad, D), F32)
    E_pad[: cfg.V] = entity_emb

    fast_relu = bool(np.all(scale == 1.0) and np.all(bias == 0.0))
    if not fast_relu:
        return _host_prep_general(cfg, x, A, Rm, relpad, E_pad, scale, bias)

    # fp8 scale calibration: score = aggr @ E^T bounded by row-norm products.
    aggr_h = A @ x + Rm @ relpad
    sbound = np.linalg.norm(aggr_h, axis=1).max() * \
        np.linalg.norm(E_pad, axis=1).max()
    ssc = F8_MAX * F8_MARGIN / max(sbound, 1e-30)
    esc = F8_MAX * F8_MARGIN / max(np.abs(E_pad).max(), 1e-30)
    asc = F8_MAX * 0.95 / max(np.abs(aggr_h).max(), 1e-30)
    psc = F8_MAX * 0.95 / max(np.abs(x).max(), np.abs(relpad).max(), 1e-30)
    col = lambda v: np.full((128, 1), v, F32)

    # shared (replicated) tensors
    ett8 = np.ascontiguousarray((E_pad.T * esc)).astype(F8)        # [128, VPAD]
    esw8 = np.ascontiguousarray(
        (E_pad * esc).reshape(cfg.NT, 8, 128, D).transpose(2, 0, 1, 3)
    ).astype(F8)                                                   # [128, NT, 8, 128]
    # x/rel as hi/lo fp8 planes on one shared scale psc (DoubleRow lhsT)
    x8 = np.ascontiguousarray(
        _hilo8(x, psc).reshape(cfg.NKX, 128, 2, D).transpose(1, 0, 2, 3)
    )                                                              # [128, NKX, 2, D]
    rel8 = np.ascontiguousarray(
        _hilo8(relpad, psc).reshape(cfg.NKR, 128, 2, D).transpose(1, 0, 2, 3)
    )

    shared = {
        "x8": x8, "rel8": rel8, "ett8": ett8, "esw8": esw8,
        "asc": col(asc / psc), "krelu": col(ssc / (esc * asc)),
        "descale": col(1.0 / (ssc * esc)),
    }
    in_maps = []
    for c in range(C):
        rows = slice(c * NPC, (c + 1) * NPC)
        a_pm = _pm_pack(np.ascontiguousarray(A[rows].T), cfg.NKX, NPC).astype(F8)
        r_pm = _pm_pack(np.ascontiguousarray(Rm[rows].T), cfg.NKR, NPC).astype(F8)
        m = dict(shared)
        m.update({"a_pm": a_pm, "r_pm": r_pm})
        in_maps.append(m)
    return in_maps, fast_relu


def build(cfg, fast_relu, lag=3, relu_cost=(1038.0, 1192.0), alt=True, nblk=8):
    if not fast_relu:
        return _build_general(cfg, fast_relu)

    f32, bf16, fp8 = mybir.dt.float32, mybir.dt.bfloat16, mybir.dt.float8e4
    nc = bacc.Bacc("TRN2", target_bir_lowering=False, debug=False)
    D, NPC, NG, NT = cfg.D, cfg.NPC, cfg.NG, cfg.NT

    xp_t = nc.dram_tensor("x8", [128, cfg.NKX, 2, D], fp8, kind="ExternalInput").ap()
    rp_t = nc.dram_tensor("rel8", [128, cfg.NKR, 2, D], fp8, kind="ExternalInput").ap()
    ap_t = nc.dram_tensor("a_pm", [128, cfg.NKX, NPC], fp8, kind="ExternalInput").ap()
    rr_t = nc.dram_tensor("r_pm", [128, cfg.NKR, NPC], fp8, kind="ExternalInput").ap()
    ett_t = nc.dram_tensor("ett8", [128, cfg.VPAD], fp8, kind="ExternalInput").ap()
    esw_t = nc.dram_tensor("esw8", [128, NT, 8, D], fp8, kind="ExternalInput").ap()
    asc_t = nc.dram_tensor("asc", [128, 1], f32, kind="ExternalInput").ap()
    krl_t = nc.dram_tensor("krelu", [128, 1], f32, kind="ExternalInput").ap()
    dsc_t = nc.dram_tensor("descale", [128, 1], f32, kind="ExternalInput").ap()
    out_t = nc.dram_tensor("out", [128, NPC], f32, kind="ExternalOutput").ap()

    Relu = mybir.ActivationFunctionType.Relu
    Copy = mybir.ActivationFunctionType.Copy
    DR = mybir.MatmulPerfMode.DoubleRow

    # relu schedule across ACT / DVE: strict alternation, with `flip` evenly
    # spaced DVE slots handed to the (faster) ACT to rebalance
    if isinstance(alt, int) and alt is not True:
        flip = alt
    else:
        flip = 0
    if alt:
        sched = [g % 2 for g in range(NG)]
        if flip:
            step = max(1, NG // (2 * flip))
            flipped = 0
            for g in range(1, NG, 2):
                if flipped < flip and (g // 2) % step == step - 1:
                    sched[g] = 0
                    flipped += 1
    else:
        engines = [0, 1]
        load = {e: 0.0 for e in engines}
        sched = []
        for _ in range(NG):
            e = min(engines, key=lambda i: load[i] + relu_cost[i])
            load[e] += relu_cost[e]
            sched.append(e)

    with tile.TileContext(nc) as tc:
        with (
            tc.tile_pool(name="const", bufs=1) as constp,
            tc.tile_pool(name="p1x", bufs=3) as p1x,
            tc.tile_pool(name="p1a", bufs=3) as p1a,
            tc.tile_pool(name="etab", bufs=6) as ep,
            tc.tile_pool(name="eswp", bufs=6) as sp,
            tc.tile_pool(name="scoresb", bufs=8) as scp,
            tc.tile_pool(name="psAcc", bufs=1, space="PSUM") as psAcc,
            tc.tile_pool(name="psS", bufs=3, space="PSUM") as psS,
        ):
            asc_sb = constp.tile([128, 1], f32, tag="asc")
            krl_sb = constp.tile([128, 1], f32, tag="krl")
            dsc_sb = constp.tile([128, 1], f32, tag="dsc")
            aggrT8 = constp.tile([128, 2, NPC], fp8, tag="aggrT8")
            out_sb = constp.tile([128, NPC], f32, tag="outsb")
            warm_sb = constp.tile([128, 1], fp8, tag="warm")

            # ---- phase 1: aggrT = psc * (x^T A^T + rel^T R^T), DoubleRow ---
            # lhsT planes are hi/lo fp8 of psc*x; rhs is the count matrix
            # broadcast across both planes (stride-0), so one DR matmul per
            # k-chunk computes the full-precision product.
            aggr_ps = psAcc.tile([128, NPC], f32, tag="accps")
            first = True
            for lhs_t, rhs_t, NB in ((xp_t, ap_t, cfg.NKX), (rp_t, rr_t, cfg.NKR)):
                for b0 in range(0, NB, nblk):
                    nb = min(nblk, NB - b0)
                    lt = p1x.tile([128, nb, 2, D], fp8, tag="lhs")
                    nc.sync.dma_start(lt, lhs_t[:, b0 : b0 + nb])
                    rt = p1a.tile([128, nb, NPC], fp8, tag="rhs")
                    nc.sync.dma_start(rt, rhs_t[:, b0 : b0 + nb])
                    if first:
                        # small const loads go behind the first stream block;
                        # the warm activation preloads ACT's Relu table early
                        nc.sync.dma_start(asc_sb, asc_t)
                        nc.sync.dma_start(krl_sb, krl_t)
                        nc.sync.dma_start(dsc_sb, dsc_t)
                        nc.scalar.activation(warm_sb, asc_sb, Relu)
                    for bb in range(nb):
                        last = rhs_t is rr_t and b0 + bb == NB - 1
                        nc.tensor.matmul(
                            aggr_ps,
                            lhsT=lt[:, bb],
                            rhs=rt[:, bb].unsqueeze(1).broadcast_to((128, 2, NPC)),
                            start=first, stop=last,
                            perf_mode=DR, skip_group_check=True,
                        )
                        first = False

            # aggrT8 = hi/lo fp8 split of asc*aggrT (one shared scale, so the
            # DoubleRow planes can be summed raw by the PE); both ops on DVE
            # back-to-back to avoid a cross-engine semaphore hop
            nc.vector.tensor_scalar_mul(aggrT8[:, 0, :], aggr_ps, asc_sb)
            nc.vector.scalar_tensor_tensor(
                aggrT8[:, 1, :], aggr_ps, asc_sb, aggrT8[:, 0, :],
                op0=mybir.AluOpType.mult, op1=mybir.AluOpType.subtract,
            )

            # ---- phase 2: fused double matmul over entity chunks ----------
            outT_ps = psAcc.tile([128, NPC], f32, tag="outps")
            pend = []  # software-pipelined DoubleRow mm2s: (g, esw_tile, st8)

            def flush_mm2():
                g, esw_g, st8_g = pend.pop(0)
                nc.tensor.matmul(
                    outT_ps, lhsT=esw_g, rhs=st8_g,
                    start=(g == 0), stop=(g == NG - 1),
                    perf_mode=DR, skip_group_check=True,
                )

            for tb2 in range((NT + 1) // 2):
                tbw = min(2, NT - 2 * tb2)  # tb-tiles in this DMA block
                ett_tile = ep.tile([128, tbw * 1024], fp8, tag="et")
                nc.sync.dma_start(
                    ett_tile, ett_t[:, tb2 * 2048 : tb2 * 2048 + tbw * 1024]
                )
                esw_tile = sp.tile([128, tbw, 8, D], fp8, tag="ee")
                nc.sync.dma_start(esw_tile, esw_t[:, 2 * tb2 : 2 * tb2 + tbw])
                for u in range(4 * tbw):
                    g = 8 * tb2 + u
                    sps = psS.tile([128, 1024], f32, tag="sps")
                    for h in range(2):
                        nc.tensor.matmul(
                            sps[:, h * NPC : (h + 1) * NPC],
                            lhsT=ett_tile[:, (2 * u + h) * 128 : (2 * u + h + 1) * 128]
                                .unsqueeze(1).broadcast_to((128, 2, 128)),
                            rhs=aggrT8,
                            start=True, stop=True,
                            perf_mode=DR, skip_group_check=True,
                        )
                    st8 = scp.tile([128, 2, NPC], fp8, tag="st")
                    if sched[g] == 0:
                        nc.scalar.activation(st8, sps, Relu, scale=krl_sb)
                    else:
                        nc.vector.tensor_scalar(
                            st8, sps, krl_sb, 0.0,
                            op0=mybir.AluOpType.mult, op1=mybir.AluOpType.max,
                        )
                    pend.append(
                        (g, esw_tile[:, u // 4, 2 * (u % 4) : 2 * (u % 4) + 2, :], st8)
                    )
                    if len(pend) > lag:
                        flush_mm2()
            while pend:
                flush_mm2()

            nc.scalar.activation(out_sb, outT_ps, Copy, scale=dsc_sb)
            nc.sync.dma_start(out_t, out_sb)

    nc.compile()
    return nc


# ---------------------------------------------------------------------------
# General-path fallback (non-trivial scale/bias): previous bf16 kernel.
# ---------------------------------------------------------------------------

def _host_prep_general(cfg, x, A, Rm, relpad, E_pad, scale, bias):
    vpad = cfg.VPAD
    et_tab = np.ascontiguousarray(E_pad.T).astype(BF16)            # [128, VPAD]
    e_sw = np.ascontiguousarray(
        E_pad.reshape(vpad // 512, 4, 128, cfg.D).transpose(0, 2, 1, 3)
    ).astype(BF16)
    scale_pad = np.ones(vpad, F32)
    scale_pad[: cfg.V] = scale
    bias_pad = np.zeros(vpad, F32)
    bias_pad[: cfg.V] = bias
    scaleT = np.ascontiguousarray(scale_pad.reshape(cfg.NV, 128).T)
    biasT = np.ascontiguousarray(bias_pad.reshape(cfg.NV, 128).T)

    xb = x.astype(BF16)
    rb = relpad.astype(BF16)

    shared = {
        "x_b": xb, "rel_b": rb, "et_tab": et_tab, "e_sw": e_sw,
        "scaleT": scaleT, "biasT": biasT,
    }
    in_maps = []
    for c in range(cfg.C):
        rows = slice(c * cfg.NPC, (c + 1) * cfg.NPC)
        at_c = np.ascontiguousarray(A[rows].T).astype(BF16)
        rt_c = np.ascontiguousarray(Rm[rows].T).astype(BF16)
        m = dict(shared)
        m.update({"a_t": at_c, "r_t": rt_c})
        in_maps.append(m)
    return in_maps, False


def _build_general(cfg, fast_relu):
    f32, bf16 = mybir.dt.float32, mybir.dt.bfloat16
    nc = bacc.Bacc("TRN2", target_bir_lowering=False, debug=False)
    D, NPC, NV = cfg.D, cfg.NPC, cfg.NV

    xb_t = nc.dram_tensor("x_b", [cfg.N, D], bf16, kind="ExternalInput").ap()
    rb_t = nc.dram_tensor("rel_b", [cfg.RPAD, D], bf16, kind="ExternalInput").ap()
    at_t = nc.dram_tensor("a_t", [cfg.N, NPC], bf16, kind="ExternalInput").ap()
    rt_t = nc.dram_tensor("r_t", [cfg.RPAD, NPC], bf16, kind="ExternalInput").ap()
    ett_t = nc.dram_tensor("et_tab", [128, cfg.VPAD], bf16, kind="ExternalInput").ap()
    esw_t = nc.dram_tensor("e_sw", [cfg.VPAD // 512, 128, 4, D], bf16, kind="ExternalInput").ap()
    scl_t = nc.dram_tensor("scaleT", [128, NV], f32, kind="ExternalInput").ap()
    bia_t = nc.dram_tensor("biasT", [128, NV], f32, kind="ExternalInput").ap()
    out_t = nc.dram_tensor("out", [128, NPC], f32, kind="ExternalOutput").ap()

    Relu = mybir.ActivationFunctionType.Relu

    with tile.TileContext(nc) as tc:
        with (
            tc.tile_pool(name="const", bufs=1) as constp,
            tc.tile_pool(name="aggk", bufs=4) as akp,
            tc.tile_pool(name="etab", bufs=6) as ep,
            tc.tile_pool(name="scoresb", bufs=6) as scp,
            tc.tile_pool(name="psA", bufs=1, space="PSUM") as psA,
            tc.tile_pool(name="psS", bufs=6, space="PSUM") as psS,
            tc.tile_pool(name="psO", bufs=1, space="PSUM") as psO,
        ):
            sclt = constp.tile([128, NV], f32, tag="sc")
            nc.sync.dma_start(sclt, scl_t)
            biat = constp.tile([128, NV], f32, tag="bi")
            nc.sync.dma_start(biat, bia_t)
            aggrT_sb = constp.tile([128, NPC], bf16, tag="aggrT")
            out_sb = constp.tile([128, NPC], f32, tag="outsb")

            aggr_ps = psA.tile([128, NPC], f32, tag="aggrps")
            for k in range(cfg.NKX):
                ks = slice(k * 128, (k + 1) * 128)
                xk = akp.tile([128, D], bf16, tag="lhs")
                nc.sync.dma_start(xk, xb_t[ks, :])
                ak = akp.tile([128, NPC], bf16, tag="rhs")
                nc.sync.dma_start(ak, at_t[ks, :])
                nc.tensor.matmul(
                    aggr_ps, lhsT=xk, rhs=ak,
                    start=(k == 0), stop=False, skip_group_check=True,
                )
            for k in range(cfg.NKR):
                ks = slice(k * 128, (k + 1) * 128)
                rk = akp.tile([128, D], bf16, tag="lhs")
                nc.sync.dma_start(rk, rb_t[ks, :])
                rrk = akp.tile([128, NPC], bf16, tag="rhs")
                nc.sync.dma_start(rrk, rt_t[ks, :])
                nc.tensor.matmul(
                    aggr_ps, lhsT=rk, rhs=rrk,
                    start=False, stop=(k == cfg.NKR - 1), skip_group_check=True,
                )
            nc.vector.tensor_copy(aggrT_sb, aggr_ps)

            outT_ps = psO.tile([128, NPC], f32, tag="outps")
            for vb in range(cfg.VPAD // 512):
                ett = ep.tile([128, 512], bf16, tag="et")
                nc.sync.dma_start(ett, ett_t[:, vb * 512 : (vb + 1) * 512])
                esw = ep.tile([128, 4, D], bf16, tag="ee")
                nc.sync.dma_start(esw, esw_t[vb])
                for j in range(4):
                    v = vb * 4 + j
                    sps = psS.tile([128, NPC], f32, tag="sps")
                    nc.tensor.matmul(
                        sps, lhsT=ett[:, j * 128 : (j + 1) * 128], rhs=aggrT_sb,
                        start=True, stop=True, skip_group_check=True,
                    )
                    st_sb = scp.tile([128, NPC], bf16, tag="st")
                    nc.scalar.activation(
                        st_sb, sps, Relu,
                        bias=biat[:, v : v + 1], scale=sclt[:, v : v + 1],
                    )
                    nc.tensor.matmul(
                        outT_ps, lhsT=esw[:, j, :], rhs=st_sb,
                        start=(v == 0), stop=(v == NV - 1), skip_group_check=True,
                    )

            nc.vector.tensor_copy(out_sb, outT_ps)
            nc.sync.dma_start(out_t, out_sb)

    nc.compile()
    return nc


def run(inputs, trace=False, cfg=None, **build_kwargs):
    if cfg is None:
        cfg = Cfg()
    in_maps, fast_relu = host_prep(cfg, **inputs)
    nc = build(cfg, fast_relu, **build_kwargs)
    try:
        res = run_bass_kernel_spmd(
            nc, in_maps, core_ids=list(range(cfg.C)), trace=trace,
        )
    except ModuleNotFoundError:
        # NTFF profiling hook unavailable in this container; run untraced.
        res = run_bass_kernel_spmd(
            nc, in_maps, core_ids=list(range(cfg.C)), trace=False,
        )
    outs = []
    for c in range(cfg.C):
        outs.append(np.ascontiguousarray(np.asarray(res.results[c]["out"]).T))
    full = np.concatenate(outs, axis=0).astype(np.float32)
    return full, res


def kernel(**inputs):
    full, _ = run(inputs, trace=False)
    return full


# revision 36
# speedup vs baseline: 1.6591x; 1.0324x over previous
"""Trainium2 Bass kernel for LMPNN-style GNN message passing + entity double-matmul.

Reference computation:
    msg      = (x[src] + rel_emb[rel]) * (1 - 2*neg)        # [E, D]
    aggr_out = segment_sum(msg, dst, N)                     # [N, D]
    aggr     = 0.1*x + aggr_out
    score    = relu((aggr @ E^T) * scale + bias)            # [N, V]
    out      = score @ E                                    # [N, D]

Strategy (8 NeuronCores, node-sharded, no collectives):
  * Core c owns nodes [c*512, (c+1)*512).
  * Message passing is re-expressed densely:  aggr = A @ x + R @ rel_emb,
    where A[n, m] = sum of (1-2*neg) over edges m->n  (+0.1 on the diagonal
    for the residual term) and R[n, r] the same per relation. The count
    matrices are small integers (|count| <= ~6) so they are streamed in
    fp8e4 (exact), halving phase-1 HBM traffic; x/rel_emb stay bf16 as the
    stationary matmul operands.
  * Double matmul: mm1 (score^T = ssc*E @ aggr^T) runs in bf16; relu converts
    PSUM fp32 -> fp8e4 SBUF tiles shaped [128, 2, 512]; mm2 accumulates
    out^T += (esc*E)^T @ score8 using fp8 DoubleRow matmuls (256-entity
    contraction per matmul). Scales ssc/esc are folded into the entity-table
    copies on the host (calibrated from row norms so fp8 can't overflow);
    the device divides them back out in the final PSUM->SBUF copy.
  * Relu work is spread across ACT/DVE(/GPSIMD) with a weighted schedule and
    the dependent mm2 is software-pipelined LAG groups behind mm1 so the
    TensorEngine never waits on relu latency.
  * Output is outT [128, 512] fp32 per core; host transposes/concats.
"""

import sys

import numpy as np

try:
    import concourse.bass as bass
except ImportError:  # pragma: no cover
    sys.path.insert(0, "/opt/trn_rl_repo")
    import concourse.bass as bass

import ml_dtypes

import concourse.bacc as bacc
import concourse.mybir as mybir
import concourse.tile as tile
from concourse.bass_utils import run_bass_kernel_spmd

BF16 = ml_dtypes.bfloat16
F8 = ml_dtypes.float8_e4m3  # matches mybir.dt.float8e4 (max finite 240)
F32 = np.float32

F8_MAX = 240.0
F8_MARGIN = 0.98


class Cfg:
    def __init__(self, N=4096, E=262144, D=128, R=1000, V=50000, C=8):
        self.N, self.E, self.D, self.R, self.V, self.C = N, E, D, R, V, C
        self.NPC = N // C                       # nodes per core
        assert self.NPC % 128 == 0 and N % 128 == 0
        self.RPAD = ((R + 127) // 128) * 128    # padded relation count
        self.VPAD = ((V + 1023) // 1024) * 1024  # padded entity count
        self.NV = self.VPAD // 128              # 128-entity chunks
        self.NG = self.VPAD // 256              # DoubleRow groups
        self.NT = self.VPAD // 1024             # 1024-entity stream tiles
        self.NKX = N // 128                     # k-chunks for A @ x
        self.NKR = self.RPAD // 128             # k-chunks for R @ rel
        self.XB = self.NKX // 2                 # x DMA blocks (2 chunks each)
        self.RB = self.NKR // 2


def _pm_pack(mat, nblk, width):
    """[nblk*128, width] -> partition-major [128, nblk, width]."""
    r = mat.reshape(nblk, 128, width).transpose(1, 0, 2)
    return np.ascontiguousarray(r)


def _hilo8(mat, sc):
    """fp8 hi/lo split of sc*mat -> [rows, 2, cols] (one shared scale)."""
    s = (mat * sc).astype(F32)
    hi = s.astype(F8)
    lo = (s - hi.astype(F32)).astype(F8)
    return np.stack([hi, lo], axis=1)


def host_prep(cfg, x, edge_index, relation_id, neg_flag, rel_emb, entity_emb,
              scale, bias):
    """Build per-core in_maps. The host converts the edge/index tensors into
    dense count matrices, lays tensors out partition-major for wide DMA
    descriptors, and calibrates the fp8 scales; the FP pipeline on the
    embeddings runs on device."""
    src = np.asarray(edge_index[0]).astype(np.int64)
    dst = np.asarray(edge_index[1]).astype(np.int64)
    rel = np.asarray(relation_id).astype(np.int64)
    neg = np.asarray(neg_flag).astype(np.int64)
    x = np.asarray(x, F32)
    rel_emb = np.asarray(rel_emb, F32)
    entity_emb = np.asarray(entity_emb, F32)
    scale = np.asarray(scale, F32)
    bias = np.asarray(bias, F32)

    C, NPC, D = cfg.C, cfg.NPC, cfg.D
    negc = (1.0 - 2.0 * neg).astype(F32)

    # dense message-passing operators (index preprocessing)
    A = np.zeros((cfg.N, cfg.N), F32)
    np.add.at(A, (dst, src), negc)
    A[np.arange(cfg.N), np.arange(cfg.N)] += 0.1          # residual 0.1*x
    Rm = np.zeros((cfg.N, cfg.RPAD), F32)
    np.add.at(Rm, (dst, rel), negc)

    relpad = np.zeros((cfg.RPAD, D), F32)
    relpad[: cfg.R] = rel_emb

    vpad = cfg.VPAD
    E_pad = np.zeros((vpad, D), F32)
    E_pad[: cfg.V] = entity_emb

    fast_relu = bool(np.all(scale == 1.0) and np.all(bias == 0.0))
    if not fast_relu:
        return _host_prep_general(cfg, x, A, Rm, relpad, E_pad, scale, bias)

    # fp8 scale calibration: score = aggr @ E^T bounded by row-norm products.
    aggr_h = A @ x + Rm @ relpad
    sbound = np.linalg.norm(aggr_h, axis=1).max() * \
        np.linalg.norm(E_pad, axis=1).max()
    ssc = F8_MAX * F8_MARGIN / max(sbound, 1e-30)
    esc = F8_MAX * F8_MARGIN / max(np.abs(E_pad).max(), 1e-30)
    asc = F8_MAX * 0.95 / max(np.abs(aggr_h).max(), 1e-30)
    psc = F8_MAX * 0.95 / max(np.abs(x).max(), np.abs(relpad).max(), 1e-30)
    col = lambda v: np.full((128, 1), v, F32)

    # shared (replicated) tensors
    ett8 = np.ascontiguousarray((E_pad.T * esc)).astype(F8)        # [128, VPAD]
    esw8 = np.ascontiguousarray(
        (E_pad * esc).reshape(cfg.NT, 8, 128, D).transpose(2, 0, 1, 3)
    ).astype(F8)                                                   # [128, NT, 8, 128]
    # x/rel as hi/lo fp8 planes on one shared scale psc (DoubleRow lhsT)
    x8 = np.ascontiguousarray(
        _hilo8(x, psc).reshape(cfg.NKX, 128, 2, D).transpose(1, 0, 2, 3)
    )                                                              # [128, NKX, 2, D]
    rel8 = np.ascontiguousarray(
        _hilo8(relpad, psc).reshape(cfg.NKR, 128, 2, D).transpose(1, 0, 2, 3)
    )

    shared = {
        "x8": x8, "rel8": rel8, "ett8": ett8, "esw8": esw8,
        "asc": col(asc / psc), "krelu": col(ssc / (esc * asc)),
        "descale": col(1.0 / (ssc * esc)),
    }
    in_maps = []
    for c in range(C):
        rows = slice(c * NPC, (c + 1) * NPC)
        a_pm = _pm_pack(np.ascontiguousarray(A[rows].T), cfg.NKX, NPC).astype(F8)
        r_pm = _pm_pack(np.ascontiguousarray(Rm[rows].T), cfg.NKR, NPC).astype(F8)
        m = dict(shared)
        m.update({"a_pm": a_pm, "r_pm": r_pm})
        in_maps.append(m)
    return in_maps, fast_relu


def build(cfg, fast_relu, lag=3, relu_cost=(1038.0, 1192.0), alt=True, nblk=16,
          xflip=0, flip_dist=1):
    if not fast_relu:
        return _build_general(cfg, fast_relu)

    f32, bf16, fp8 = mybir.dt.float32, mybir.dt.bfloat16, mybir.dt.float8e4
    nc = bacc.Bacc("TRN2", target_bir_lowering=False, debug=False)
    D, NPC, NG, NT = cfg.D, cfg.NPC, cfg.NG, cfg.NT

    xp_t = nc.dram_tensor("x8", [128, cfg.NKX, 2, D], fp8, kind="ExternalInput").ap()
    rp_t = nc.dram_tensor("rel8", [128, cfg.NKR, 2, D], fp8, kind="ExternalInput").ap()
    ap_t = nc.dram_tensor("a_pm", [128, cfg.NKX, NPC], fp8, kind="ExternalInput").ap()
    rr_t = nc.dram_tensor("r_pm", [128, cfg.NKR, NPC], fp8, kind="ExternalInput").ap()
    ett_t = nc.dram_tensor("ett8", [128, cfg.VPAD], fp8, kind="ExternalInput").ap()
    esw_t = nc.dram_tensor("esw8", [128, NT, 8, D], fp8, kind="ExternalInput").ap()
    asc_t = nc.dram_tensor("asc", [128, 1], f32, kind="ExternalInput").ap()
    krl_t = nc.dram_tensor("krelu", [128, 1], f32, kind="ExternalInput").ap()
    dsc_t = nc.dram_tensor("descale", [128, 1], f32, kind="ExternalInput").ap()
    out_t = nc.dram_tensor("out", [128, NPC], f32, kind="ExternalOutput").ap()

    Relu = mybir.ActivationFunctionType.Relu
    Copy = mybir.ActivationFunctionType.Copy
    DR = mybir.MatmulPerfMode.DoubleRow

    # relu schedule across ACT / DVE: strict alternation, with `flip` evenly
    # spaced DVE slots handed to the (faster) ACT to rebalance
    if isinstance(alt, int) and alt is not True:
        flip = alt
    else:
        flip = 0
    if alt:
        sched = [g % 2 for g in range(NG)]
        if flip:
            step = max(1, NG // (2 * flip))
            flipped = 0
            for g in range(1, NG, 2):
                if flipped < flip and (g // 2) % step == step - 1:
                    sched[g] = 0
                    flipped += 1
    else:
        engines = [0, 1]
        load = {e: 0.0 for e in engines}
        sched = []
        for _ in range(NG):
            e = min(engines, key=lambda i: load[i] + relu_cost[i])
            load[e] += relu_cost[e]
            sched.append(e)

    with tile.TileContext(nc) as tc:
        with (
            tc.tile_pool(name="const", bufs=1) as constp,
            tc.tile_pool(name="p1x", bufs=3) as p1x,
            tc.tile_pool(name="p1a", bufs=3) as p1a,
            tc.tile_pool(name="etab", bufs=6) as ep,
            tc.tile_pool(name="eswp", bufs=6) as sp,
            tc.tile_pool(name="scoresb", bufs=8) as scp,
            tc.tile_pool(name="psAcc", bufs=1, space="PSUM") as psAcc,
            tc.tile_pool(name="psS", bufs=3, space="PSUM") as psS,
            tc.tile_pool(name="psX", bufs=1, space="PSUM") as psX,
        ):
            asc_sb = constp.tile([128, 1], f32, tag="asc")
            krl_sb = constp.tile([128, 1], f32, tag="krl")
            dsc_sb = constp.tile([128, 1], f32, tag="dsc")
            aggrT8 = constp.tile([128, 2, NPC], fp8, tag="aggrT8")
            out_sb = constp.tile([128, NPC], f32, tag="outsb")
            warm_sb = constp.tile([128, 1], fp8, tag="warm")

            # ---- phase 1: aggrT = psc * (x^T A^T + rel^T R^T), DoubleRow ---
            # lhsT planes are hi/lo fp8 of psc*x; rhs is the count matrix
            # broadcast across both planes (stride-0), so one DR matmul per
            # k-chunk computes the full-precision product.
            aggr_ps = psAcc.tile([128, NPC], f32, tag="acc", name="aggr_ps")
            first = True
            for lhs_t, rhs_t, NB in ((xp_t, ap_t, cfg.NKX), (rp_t, rr_t, cfg.NKR)):
                for b0 in range(0, NB, nblk):
                    nb = min(nblk, NB - b0)
                    lt = p1x.tile([128, nb, 2, D], fp8, tag="lhs")
                    nc.sync.dma_start(lt, lhs_t[:, b0 : b0 + nb])
                    rt = p1a.tile([128, nb, NPC], fp8, tag="rhs")
                    nc.sync.dma_start(rt, rhs_t[:, b0 : b0 + nb])
                    if first:
                        # small const loads go behind the first stream block;
                        # the warm activation preloads ACT's Relu table early
                        nc.sync.dma_start(asc_sb, asc_t)
                        nc.sync.dma_start(krl_sb, krl_t)
                        nc.sync.dma_start(dsc_sb, dsc_t)
                        nc.scalar.activation(warm_sb, asc_sb, Relu)
                    for bb in range(nb):
                        last = rhs_t is rr_t and b0 + bb == NB - 1
                        nc.tensor.matmul(
                            aggr_ps,
                            lhsT=lt[:, bb],
                            rhs=rt[:, bb].unsqueeze(1).broadcast_to((128, 2, NPC)),
                            start=first, stop=last,
                            perf_mode=DR, skip_group_check=True,
                        )
                        first = False

            # aggrT8 = hi/lo fp8 split of asc*aggrT (one shared scale, so the
            # DoubleRow planes can be summed raw by the PE); both ops on DVE
            # back-to-back to avoid a cross-engine semaphore hop
            nc.vector.tensor_scalar_mul(aggrT8[:, 0, :], aggr_ps, asc_sb)
            nc.vector.scalar_tensor_tensor(
                aggrT8[:, 1, :], aggr_ps, asc_sb, aggrT8[:, 0, :],
                op0=mybir.AluOpType.mult, op1=mybir.AluOpType.subtract,
            )

            # ---- phase 2: fused double matmul over entity chunks ----------
            # `xflip` DVE-parity groups are offloaded to ACT through the spare
            # 8th PSUM bank (psX); their two half-chunks are interleaved with
            # the following group's matmuls so the in-order PE never blocks on
            # ACT's relu latency.
            flip_set = set()
            if alt and xflip:
                lo, hi = 16, NG - 16
                for i in range(xflip):
                    g = lo + (2 * i + 1) * (hi - lo) // (2 * xflip)
                    flip_set.add(g | 1)  # force DVE parity
            outT_ps = psAcc.tile([128, NPC], f32, tag="acc", name="outT_ps")
            pend = []  # software-pipelined DoubleRow mm2s: (g, esw_tile, st8)

            def flush_mm2():
                g, esw_g, st8_g = pend.pop(0)
                nc.tensor.matmul(
                    outT_ps, lhsT=esw_g, rhs=st8_g,
                    start=(g == 0), stop=(g == NG - 1),
                    perf_mode=DR, skip_group_check=True,
                )

            def emit_flip_half(fl, h):
                g, lhs_lo, lhs_hi, esw_g, st8 = fl
                lhs = (lhs_lo, lhs_hi)[h]
                spx = psX.tile([128, NPC], f32, tag="spx")
                nc.tensor.matmul(
                    spx, lhsT=lhs, rhs=aggrT8,
                    start=True, stop=True, perf_mode=DR, skip_group_check=True,
                )
                nc.scalar.activation(st8[:, h, :], spx, Relu, scale=krl_sb)
                if h == 1:
                    pend.append((g, esw_g, st8))

            flip_pend = None
            flip_wait = 0
            for tb2 in range((NT + 1) // 2):
                tbw = min(2, NT - 2 * tb2)  # tb-tiles in this DMA block
                ett_tile = ep.tile([128, tbw * 1024], fp8, tag="et")
                nc.sync.dma_start(
                    ett_tile, ett_t[:, tb2 * 2048 : tb2 * 2048 + tbw * 1024]
                )
                esw_tile = sp.tile([128, tbw, 8, D], fp8, tag="ee")
                nc.sync.dma_start(esw_tile, esw_t[:, 2 * tb2 : 2 * tb2 + tbw])
                for u in range(4 * tbw):
                    g = 8 * tb2 + u
                    lhsT_of = lambda h: (
                        ett_tile[:, (2 * u + h) * 128 : (2 * u + h + 1) * 128]
                        .unsqueeze(1).broadcast_to((128, 2, 128))
                    )
                    esw_g = esw_tile[:, u // 4, 2 * (u % 4) : 2 * (u % 4) + 2, :]
                    if g in flip_set:
                        st8 = scp.tile([128, 2, NPC], fp8, tag="st")
                        flip_pend = (g, lhsT_of(0), lhsT_of(1), esw_g, st8)
                        flip_wait = flip_dist
                        emit_flip_half(flip_pend, 0)
                        continue
                    sps = psS.tile([128, 1024], f32, tag="sps")
                    for h in range(2):
                        nc.tensor.matmul(
                            sps[:, h * NPC : (h + 1) * NPC],
                            lhsT=lhsT_of(h), rhs=aggrT8,
                            start=True, stop=True,
                            perf_mode=DR, skip_group_check=True,
                        )
                    if flip_pend is not None:
                        flip_wait -= 1
                        if flip_wait <= 0:
                            emit_flip_half(flip_pend, 1)
                            flip_pend = None
                    st8 = scp.tile([128, 2, NPC], fp8, tag="st")
                    if sched[g] == 0:
                        nc.scalar.activation(st8, sps, Relu, scale=krl_sb)
                    else:
                        nc.vector.tensor_scalar(
                            st8, sps, krl_sb, 0.0,
                            op0=mybir.AluOpType.mult, op1=mybir.AluOpType.max,
                        )
                    pend.append((g, esw_g, st8))
                    if len(pend) > lag:
                        flush_mm2()
            if flip_pend is not None:
                emit_flip_half(flip_pend, 1)
                flip_pend = None
            while pend:
                flush_mm2()

            nc.scalar.activation(out_sb, outT_ps, Copy, scale=dsc_sb)
            nc.sync.dma_start(out_t, out_sb)

    nc.compile()
    return nc


# ---------------------------------------------------------------------------
# General-path fallback (non-trivial scale/bias): previous bf16 kernel.
# ---------------------------------------------------------------------------

def _host_prep_general(cfg, x, A, Rm, relpad, E_pad, scale, bias):
    vpad = cfg.VPAD
    et_tab = np.ascontiguousarray(E_pad.T).astype(BF16)            # [128, VPAD]
    e_sw = np.ascontiguousarray(
        E_pad.reshape(vpad // 512, 4, 128, cfg.D).transpose(0, 2, 1, 3)
    ).astype(BF16)
    scale_pad = np.ones(vpad, F32)
    scale_pad[: cfg.V] = scale
    bias_pad = np.zeros(vpad, F32)
    bias_pad[: cfg.V] = bias
    scaleT = np.ascontiguousarray(scale_pad.reshape(cfg.NV, 128).T)
    biasT = np.ascontiguousarray(bias_pad.reshape(cfg.NV, 128).T)

    xb = x.astype(BF16)
    rb = relpad.astype(BF16)

    shared = {
        "x_b": xb, "rel_b": rb, "et_tab": et_tab, "e_sw": e_sw,
        "scaleT": scaleT, "biasT": biasT,
    }
    in_maps = []
    for c in range(cfg.C):
        rows = slice(c * cfg.NPC, (c + 1) * cfg.NPC)
        at_c = np.ascontiguousarray(A[rows].T).astype(BF16)
        rt_c = np.ascontiguousarray(Rm[rows].T).astype(BF16)
        m = dict(shared)
        m.update({"a_t": at_c, "r_t": rt_c})
        in_maps.append(m)
    return in_maps, False


def _build_general(cfg, fast_relu):
    f32, bf16 = mybir.dt.float32, mybir.dt.bfloat16
    nc = bacc.Bacc("TRN2", target_bir_lowering=False, debug=False)
    D, NPC, NV = cfg.D, cfg.NPC, cfg.NV

    xb_t = nc.dram_tensor("x_b", [cfg.N, D], bf16, kind="ExternalInput").ap()
    rb_t = nc.dram_tensor("rel_b", [cfg.RPAD, D], bf16, kind="ExternalInput").ap()
    at_t = nc.dram_tensor("a_t", [cfg.N, NPC], bf16, kind="ExternalInput").ap()
    rt_t = nc.dram_tensor("r_t", [cfg.RPAD, NPC], bf16, kind="ExternalInput").ap()
    ett_t = nc.dram_tensor("et_tab", [128, cfg.VPAD], bf16, kind="ExternalInput").ap()
    esw_t = nc.dram_tensor("e_sw", [cfg.VPAD // 512, 128, 4, D], bf16, kind="ExternalInput").ap()
    scl_t = nc.dram_tensor("scaleT", [128, NV], f32, kind="ExternalInput").ap()
    bia_t = nc.dram_tensor("biasT", [128, NV], f32, kind="ExternalInput").ap()
    out_t = nc.dram_tensor("out", [128, NPC], f32, kind="ExternalOutput").ap()

    Relu = mybir.ActivationFunctionType.Relu

    with tile.TileContext(nc) as tc:
        with (
            tc.tile_pool(name="const", bufs=1) as constp,
            tc.tile_pool(name="aggk", bufs=4) as akp,
            tc.tile_pool(name="etab", bufs=6) as ep,
            tc.tile_pool(name="scoresb", bufs=6) as scp,
            tc.tile_pool(name="psA", bufs=1, space="PSUM") as psA,
            tc.tile_pool(name="psS", bufs=6, space="PSUM") as psS,
            tc.tile_pool(name="psO", bufs=1, space="PSUM") as psO,
        ):
            sclt = constp.tile([128, NV], f32, tag="sc")
            nc.sync.dma_start(sclt, scl_t)
            biat = constp.tile([128, NV], f32, tag="bi")
            nc.sync.dma_start(biat, bia_t)
            aggrT_sb = constp.tile([128, NPC], bf16, tag="aggrT")
            out_sb = constp.tile([128, NPC], f32, tag="outsb")

            aggr_ps = psA.tile([128, NPC], f32, tag="aggrps")
            for k in range(cfg.NKX):
                ks = slice(k * 128, (k + 1) * 128)
                xk = akp.tile([128, D], bf16, tag="lhs")
                nc.sync.dma_start(xk, xb_t[ks, :])
                ak = akp.tile([128, NPC], bf16, tag="rhs")
                nc.sync.dma_start(ak, at_t[ks, :])
                nc.tensor.matmul(
                    aggr_ps, lhsT=xk, rhs=ak,
                    start=(k == 0), stop=False, skip_group_check=True,
                )
            for k in range(cfg.NKR):
                ks = slice(k * 128, (k + 1) * 128)
                rk = akp.tile([128, D], bf16, tag="lhs")
                nc.sync.dma_start(rk, rb_t[ks, :])
                rrk = akp.tile([128, NPC], bf16, tag="rhs")
                nc.sync.dma_start(rrk, rt_t[ks, :])
                nc.tensor.matmul(
                    aggr_ps, lhsT=rk, rhs=rrk,
                    start=False, stop=(k == cfg.NKR - 1), skip_group_check=True,
                )
            nc.vector.tensor_copy(aggrT_sb, aggr_ps)

            outT_ps = psO.tile([128, NPC], f32, tag="outps")
            for vb in range(cfg.VPAD // 512):
                ett = ep.tile([128, 512], bf16, tag="et")
                nc.sync.dma_start(ett, ett_t[:, vb * 512 : (vb + 1) * 512])
                esw = ep.tile([128, 4, D], bf16, tag="ee")
                nc.sync.dma_start(esw, esw_t[vb])
                for j in range(4):
                    v = vb * 4 + j
                    sps = psS.tile([128, NPC], f32, tag="sps")
                    nc.tensor.matmul(
                        sps, lhsT=ett[:, j * 128 : (j + 1) * 128], rhs=aggrT_sb,
                        start=True, stop=True, skip_group_check=True,
                    )
                    st_sb = scp.tile([128, NPC], bf16, tag="st")
                    nc.scalar.activation(
                        st_sb, sps, Relu,
                        bias=biat[:, v : v + 1], scale=sclt[:, v : v + 1],
                    )
                    nc.tensor.matmul(
                        outT_ps, lhsT=esw[:, j, :], rhs=st_sb,
                        start=(v == 0), stop=(v == NV - 1), skip_group_check=True,
                    )

            nc.vector.tensor_copy(out_sb, outT_ps)
            nc.sync.dma_start(out_t, out_sb)

    nc.compile()
    return nc


def run(inputs, trace=False, cfg=None, **build_kwargs):
    if cfg is None:
        cfg = Cfg()
    in_maps, fast_relu = host_prep(cfg, **inputs)
    nc = build(cfg, fast_relu, **build_kwargs)
    try:
        res = run_bass_kernel_spmd(
            nc, in_maps, core_ids=list(range(cfg.C)), trace=trace,
        )
    except ModuleNotFoundError:
        # NTFF profiling hook unavailable in this container; run untraced.
        res = run_bass_kernel_spmd(
            nc, in_maps, core_ids=list(range(cfg.C)), trace=False,
        )
    outs = []
    for c in range(cfg.C):
        outs.append(np.ascontiguousarray(np.asarray(res.results[c]["out"]).T))
    full = np.concatenate(outs, axis=0).astype(np.float32)
    return full, res


def kernel(**inputs):
    full, _ = run(inputs, trace=False)
    return full
